# revision 70
# baseline (speedup 1.0000x reference)
"""Trainium2 Bass kernel for nn_Decoder (worker/task label-probability decoder).

Math:
    worker_feature = inputs[:2048, :64]          # [Wn, A]
    tau            = inputs[2048:, :16]          # [T, L]
    p1 = sigmoid(worker_feature @ W + b)         # [Wn, 1]
    p2 = (1 - p1) / (L - 1)
    P[i, j, l] = p1[i]^tau[j,l] * p2[i]^(1 - tau[j,l])
               = exp(a[i] * tau[j,l] + c[i]),  a = ln p1 - ln p2, c = ln p2

Sharding: pure data parallel over the worker axis (dim 0), 256 workers per
core across 8 cores; tau/W/b replicated. No communication.

Per-core layout: workers on SBUF partitions (2 groups of 128), task axis
flattened on the free dimension. tau ships as an exact 3-term bf16 split,
striped [48 x 2048] so the load is one cheap DMA; the otherwise-idle tensor
engine replicates each stripe to all 128 partitions (ones.T @ tau3 sums the
three bf16 terms in fp32 -> exact tau in PSUM). The scalar engine computes
Exp(a*tau + c) with per-partition scale/bias, writing bf16 tiles (the 2e-2
rel-err budget dwarfs bf16's 2e-3 rounding), halving HBM write traffic.

The scalar engine is the roofline (~58 us of Exp incl. per-op overhead);
everything else is arranged to keep it saturated: the DVE drains each PSUM
piece into [128, 8192] f32 SBUF stages (so the 2-deep PSUM ring frees fast
and the PE never gates ACT) and ACT runs wide Exp ops out of SBUF; output
DMAs ride only the SP and Pool queues (a DMA costs its issuing engine
~0.39 ns per partition-byte); the Exp/Ln table is prefetched via a dummy
op during the input loads; dummy matmuls warm the PE p-state; and the
first/last stages are read in a copy-chasing ladder so pipeline fill and
drain stay short.
"""

import numpy as np

try:
    import concourse.bass as bass  # noqa: F401
except ImportError:  # fall back to the container's repo checkout
    import sys

    for _p in ("/root/.axon_site/_ro/trn_rl_repo", "/opt/trn_rl_repo"):
        if _p not in sys.path:
            sys.path.append(_p)

import concourse.bass as bass
import concourse.tile as tile
from concourse import library_config, mybir
from concourse.bass_utils import run_bass_kernel_spmd
from concourse.vector_clock import ScopedClock

WN = 2048  # workers total
TN = 2048  # tasks
L = 16  # edge types / labels
A = 64  # ability features
NCORES = 8
WPC = WN // NCORES  # workers per core (256)
G = WPC // 128  # partition groups per core (2)
F = TN * L  # flattened task axis (32768)
CH = 2048  # stripe length: PSUM ping-pong granularity + per-chunk output DMA
NST = F // CH  # tau stripes (16)
NWARM = 18  # dummy matmuls to warm the PE p-state

SW = 8192  # SBUF stage width: 4 PSUM pieces aggregated per stage
NSTG = F // SW  # stages (4)

_AF = mybir.ActivationFunctionType

# minimax-ish quadratic for 2^f on [-0.5, 0.5] (rel err ~2.4e-3)
_K0, _K1, _K2 = 1.0005544420658883, 0.7060853046731855, 0.23974148211482982
_LOG2E = 1.4426950408889634


class _TC(tile.TileContext):
    """TileContext legalized for a walrus that allows one sync-wait per inst.

    The walrus build in this container rejects any instruction carrying more
    than one sync-wait command. After Tile's normal scheduling + the exit
    drain/barrier, rewrite every multi-wait instruction into a chain of
    same-engine NOPs (one wait each) followed by the instruction with the
    final wait.
    """

    def _drain_and_barrier(self, tick_clock, wait_clock):
        super()._drain_and_barrier(tick_clock, wait_clock)
        self._split_multi_waits()

    def _fresh_nop(self, engine):
        inst = self.nc.engines[engine].nop(nofuse=True).ins
        self.nc.cur_bb.bb.instructions.remove(inst)
        return inst

    def _split_multi_waits(self):
        for fn in self.nc.m.functions:
            for bb in fn.blocks:
                snapshot = list(bb.instructions)
                if not any(
                    inst.sync_info and len(inst.sync_info.on_wait) > 1
                    for inst in snapshot
                ):
                    continue
                new = []
                for inst in snapshot:
                    si = inst.sync_info
                    if si is not None and si.on_wait and len(si.on_wait) > 1:
                        waits = list(si.on_wait)
                        si.on_wait = waits[-1:]
                        inst.sync_info = si
                        for wt in waits[:-1]:
                            nop = self._fresh_nop(inst.engine)
                            nop.sync_info = mybir.SyncInfo(on_wait=[wt], on_update=[])
                            new.append(nop)
                    new.append(inst)
                bb.instructions[:] = new


def build_nc():
    nc = bass.Bass("TRN2")
    wf = nc.dram_tensor("wf", [WPC, A], mybir.dt.float32, kind="ExternalInput")
    # tau3[3*s + t] = bf16 term t of tau stripe s (exact hi/mid/lo split).
    tau3_in = nc.dram_tensor(
        "tau3", [3 * NST, CH], mybir.dt.bfloat16, kind="ExternalInput"
    )
    # mask[k, s] = 1.0 iff row k belongs to stripe s. Used as matmul weights
    # via a stride-0 AP that broadcasts column s to all 128 weight columns,
    # so the contraction over all 48 rows extracts exactly stripe s's three
    # bf16 terms (their fp32 sum = tau, bit-exact).
    mask_in = nc.dram_tensor(
        "mask", [3 * NST, NST], mybir.dt.bfloat16, kind="ExternalInput"
    )
    w_in = nc.dram_tensor("W", [A], mybir.dt.float32, kind="ExternalInput")
    b_in = nc.dram_tensor("b", [1], mybir.dt.float32, kind="ExternalInput")
    out = nc.dram_tensor("out", [G, 128, F], mybir.dt.bfloat16, kind="ExternalOutput")

    f32 = mybir.dt.float32
    bf16 = mybir.dt.bfloat16

    with _TC(nc) as tc:
        with (
            tc.tile_pool(name="const", bufs=1) as const,
            tc.tile_pool(name="outs", bufs=3) as outs,
            tc.tile_pool(name="psum", bufs=2, space="PSUM") as psum,
        ):
            ones = const.tile([3, 128], bf16)
            nc.vector.memset(ones, 1.0)

            # Prefetch the Exp/Ln activation table while inputs load: the
            # first table load costs 1.28 us and must not sit on the
            # prologue's critical path.
            scr_act = const.tile([3, 128], bf16)
            nc.scalar.activation(scr_act, ones, _AF.Exp)

            # Warm the PE p-state: dummy matmuls keep the tensor engine
            # continuously busy from t~0 so the real chunk-0 matmuls run at
            # full clock instead of the 0.65 GHz cold state.
            for wi in range(NWARM):
                wpt = psum.tile([128, CH], f32, tag="pt", name=f"warm{wi}")
                nc.tensor.matmul(
                    wpt[:, 0:128], ones, ones, start=True, stop=True
                )

            # ---- input loads: wf/W/b on SP, tau3 on Pool (one 4KB/partition
            # DMA); the ACT ring never carries a DMA.
            wf_sb = const.tile([128, G, A], f32)
            nc.sync.dma_start(
                out=wf_sb, in_=wf[:].rearrange("(g p) a -> p g a", p=128)
            )
            w_ap = w_in[:]
            w_sb = const.tile([128, A], f32)
            nc.sync.dma_start(
                out=w_sb,
                in_=bass.AP(tensor=w_ap.tensor, offset=w_ap.offset, ap=[[0, 128], [1, A]]),
            )
            b_ap = b_in[:]
            b_sb = const.tile([128, 1], f32)
            nc.sync.dma_start(
                out=b_sb,
                in_=bass.AP(tensor=b_ap.tensor, offset=b_ap.offset, ap=[[0, 128], [1, 1]]),
            )
            # tau loaded in column pieces so stripe 0's early matmuls (which
            # only read the first columns) can start ~2.3us in; the mask is a
            # single tiny load.
            tau_sb = const.tile([3 * NST, CH], bf16)
            mask_sb = const.tile([3 * NST, NST], bf16)
            nc.gpsimd.dma_start(out=tau_sb[:, :512], in_=tau3_in[:, :512])
            nc.gpsimd.dma_start(out=mask_sb, in_=mask_in[:])
            nc.gpsimd.dma_start(out=tau_sb[:, 512:1024], in_=tau3_in[:, 512:1024])
            nc.gpsimd.dma_start(out=tau_sb[:, 1024:], in_=tau3_in[:, 1024:])

            def stripe_w(s):
                col = mask_sb[:, s : s + 1]
                return bass.AP(
                    tensor=col.tensor, offset=col.offset, ap=[[col.ap[0][0], 3 * NST], [0, 128]]
                )

            # ---- per-worker scalars (on GPSIMD, so the DVE is free to run
            # the PSUM->SBUF copy pipeline from t~3us):
            #   x' = wf.W + b;  a = ln p1 - ln p2 = x' + ln15  (exact identity)
            #   c  = ln p2 = -x' - ln15 - ln(1 + e^{-x'})
            LN15 = float(np.log(np.float32(L - 1)))
            x = const.tile([128, G], f32)
            for g in range(G):
                prod = const.tile([128, A], f32, tag=f"prod{g}")
                nc.vector.tensor_mul(prod, wf_sb[:, g, :], w_sb)
                nc.vector.reduce_sum(x[:, g : g + 1], prod, axis=mybir.AxisListType.X)

            pb = const.tile([128, 1], f32)  # b + ln15
            nc.vector.tensor_scalar_add(pb, b_sb, LN15)
            bneg = const.tile([128, 1], f32)
            nc.vector.tensor_scalar_mul(bneg, b_sb, -1.0)
            a_sb = const.tile([128, G], f32)
            nc.gpsimd.tensor_scalar_add(a_sb, x, pb[:, 0:1])
            # e = exp(-(x + b));  u = ln(1 + e);  c = -(u + x + b + ln15)
            e = const.tile([128, G], f32)
            nc.scalar.activation(e, x, _AF.Exp, bias=bneg[:, 0:1], scale=-1.0)
            u = const.tile([128, G], f32)
            nc.scalar.activation(u, e, _AF.Ln, bias=1.0, scale=1.0)
            s1 = const.tile([128, G], f32)
            nc.gpsimd.tensor_add(s1, u, x)
            lp2 = const.tile([128, G], f32)  # = c
            nc.gpsimd.tensor_scalar(
                lp2,
                s1,
                scalar1=pb[:, 0:1],
                scalar2=-1.0,
                op0=mybir.AluOpType.add,
                op1=mybir.AluOpType.mult,
            )
            # log2-scaled scale/bias for the DVE exp2 chunk
            ap_sb = const.tile([128, G], f32)
            nc.gpsimd.tensor_scalar_mul(ap_sb, a_sb, _LOG2E)
            cp_sb = const.tile([128, G], f32)
            nc.gpsimd.tensor_scalar_mul(cp_sb, lp2, _LOG2E)
            # DVE exp2 scratch
            xb1 = const.tile([128, CH], f32)
            xb2 = const.tile([128, CH], f32)
            xbi = const.tile([128, CH], mybir.dt.int32)
            gb1 = const.tile([128, 1024], f32)
            gb2 = const.tile([128, 1024], f32)
            gbi = const.tile([128, 1024], mybir.dt.int32)
            gb3 = const.tile([128, 1024], f32)

            # ---- main loop ----
            # PE broadcasts tau stripe s into a PSUM piece; the otherwise-idle
            # DVE drains each piece to a [128, 8192] f32 SBUF stage (freeing
            # the 2-deep PSUM ring early, so the PE never gates the scalar
            # engine); ACT then runs big 8192-wide Exp ops out of SBUF (fewer
            # per-instruction overheads) and the bf16 result streams to HBM
            # in 2048-chunks alternating between the SP and Pool queues.
            dma_engines = [nc.sync, nc.gpsimd]
            qi = 0

            # Work queue: stripe 0 split into 3 column-pieces (matching the
            # tau load ladder) so the copy pipeline starts with the first
            # 512 columns; each piece gets its own PSUM tile so a copy only
            # waits for the matmuls it actually covers.
            pieces = [(0, 0, 512), (0, 512, 512), (0, 1024, 1024)] + [
                (s, 0, CH) for s in range(1, NST)
            ]

            def stage_plan(i):
                # Stage 0: group 0 chases the copy pipeline piece-by-piece
                # (ACT's 2 passes/element outrun the copy's ~1.04
                # ns/element); group 1 follows in two 4096 reads. Last
                # stage: the final ACT is small so the drain tail is short.
                if i == 0:
                    g0 = [(0, 512), (512, 512), (1024, 1024), (2048, 2048),
                          (4096, 2048), (6144, 2048)]
                    return [(g, off, w) for off, w in g0 for g in range(G)]
                if i == 1:
                    # DVE hasn't caught up yet when ACT reaches stage 1: let
                    # the first reader chase per-piece, second reads whole.
                    return [(0, 0, CH), (0, CH, CH), (0, 2 * CH, 2 * CH)] + [(1, 1024, SW - 1024)]
                if i == 2:
                    # g1's first 1024 run on the DVE (exp2 bit-trick)
                    return [(0, 0, SW), (1, 1280, SW - 1280)]
                if i == NSTG - 1:
                    # g1's first 2048 run on the DVE (exp2 bit-trick), not ACT
                    g1 = [(1536, 2048), (3584, 2048),
                          (5632, 2048), (7680, 512)]
                    return [(0, 0, SW)] + [(1, off, w) for off, w in g1]
                return [(g, 0, SW) for g in range(G)]

            stgs, otss = {}, {}
            stg = None
            cum = 0
            for p, (s, c0, w) in enumerate(pieces):
                pt = psum.tile([128, CH], f32, tag="pt", name=f"pt{p}")
                for n in range(0, w, 512):
                    nc.tensor.matmul(
                        pt[:, n : n + 512],
                        stripe_w(s),
                        tau_sb[:, c0 + n : c0 + n + 512],
                        start=True,
                        stop=True,
                    )
                i, off = divmod(cum, SW)
                if off == 0:
                    stg = outs.tile([128, SW], f32, tag="stg", name=f"stg{i}", bufs=3)
                    stgs[i] = stg
                nc.vector.tensor_scalar_mul(
                    stg[:, off : off + w], pt[:, :w], 1.0
                )
                cum += w
                if cum % SW != 0:
                    continue
                ots = [
                    outs.tile([128, SW], bf16, tag=f"ot{g}", name=f"ot{g}_s{i}", bufs=2)
                    for g in range(G)
                ]
                otss[i] = ots
                if i == 2:
                    # GPSIMD exp2 chunk for stage1-g1[0:1024]: the 6 plain
                    # f32 ops encode on Pool; the int32-converting op and the
                    # bitcast-reading stt run on DVE (emitted post-loop).
                    AL = mybir.AluOpType
                    KM = float(1.5 * 2**23)
                    gp = nc.gpsimd
                    st1 = stgs[1]
                    gp.tensor_scalar(gb1, st1[:, :1024], ap_sb[:, 1:2],
                                     cp_sb[:, 1:2], op0=AL.mult, op1=AL.add)
                    gp.tensor_scalar(gb2, gb1, KM, None, op0=AL.add)
                    gp.tensor_scalar(gb2, gb2, -KM, None, op0=AL.add)
                    gp.tensor_sub(gb1, gb1, gb2)
                    gp.tensor_scalar(gb3, gb1, _K2, _K1, op0=AL.mult, op1=AL.add)
                    gp.tensor_mul(gb3, gb3, gb1)
                for g, off, w in stage_plan(i):
                    nc.scalar.activation(
                        ots[g][:, off : off + w],
                        stg[:, off : off + w],
                        _AF.Exp,
                        bias=lp2[:, g : g + 1],
                        scale=a_sb[:, g : g + 1],
                    )
                    for d0 in range(off, off + w, CH):
                        dw = min(CH, off + w - d0)
                        dma_engines[qi % 2].dma_start(
                            out=out[g, :, i * SW + d0 : i * SW + d0 + dw],
                            in_=ots[g][:, d0 : d0 + dw],
                        )
                        qi += 1

            # DVE exp2 bit-trick chunks + the two DVE-only hops of the
            # GPSIMD chunk (f32->int32 convert and bitcast-reading stt).
            AL = mybir.AluOpType
            KM = float(1.5 * 2**23)
            ve = nc.vector
            ve.tensor_scalar(gbi, gb2, 127.0, 8388608.0,
                             op0=AL.add, op1=AL.mult)  # GP chunk exp bits
            # wait: gb2 holds n only until GP's q overwrites it; the Tile
            # dependency on gb2's WAR serializes GP's q behind this op.
            for ci, (i, g, w) in enumerate([(2, 1, 1280)]):
                stg, ot = stgs[i], otss[i][g]
                b1, b2, bi = xb1[:, :w], xb2[:, :w], xbi[:, :w]
                ve.tensor_scalar(b1, stg[:, :w], ap_sb[:, g : g + 1],
                                 cp_sb[:, g : g + 1], op0=AL.mult, op1=AL.add)
                ve.tensor_scalar(b2, b1, KM, None, op0=AL.add)
                ve.tensor_scalar(b2, b2, -KM, None, op0=AL.add)
                ve.tensor_scalar(bi, b2, 127.0, 8388608.0,
                                 op0=AL.add, op1=AL.mult)
                ve.tensor_sub(b1, b1, b2)
                ve.tensor_scalar(b2, b1, _K2, _K1, op0=AL.mult, op1=AL.add)
                ve.tensor_mul(b2, b2, b1)
                ve.scalar_tensor_tensor(ot[:, :w], b2, _K0,
                                        bi.bitcast(f32),
                                        op0=AL.add, op1=AL.mult)
                for d0 in range(0, w, 1024):
                    dma_engines[qi % 2].dma_start(
                        out=out[g, :, i * SW + d0 : i * SW + d0 + 1024],
                        in_=ot[:, d0 : d0 + 1024],
                    )
                    qi += 1
            # GP chunk finish: (m + k0) * 2^n -> ot (stage1 g1 [0:1024])
            ve.scalar_tensor_tensor(otss[1][1][:, :1024], gb3, _K0,
                                    gbi.bitcast(f32), op0=AL.add, op1=AL.mult)
            nc.gpsimd.dma_start(
                out=out[1, :, SW : SW + 1024], in_=otss[1][1][:, :1024]
            )
            # DVE chunk1: stage3 g1 [0:1536]
            for ci, (i, g, w) in enumerate([(3, 1, 1536)]):
                stg, ot = stgs[i], otss[i][g]
                b1, b2, bi = xb1[:, :w], xb2[:, :w], xbi[:, :w]
                ve.tensor_scalar(b1, stg[:, :w], ap_sb[:, g : g + 1],
                                 cp_sb[:, g : g + 1], op0=AL.mult, op1=AL.add)
                ve.tensor_scalar(b2, b1, KM, None, op0=AL.add)
                ve.tensor_scalar(b2, b2, -KM, None, op0=AL.add)
                ve.tensor_scalar(bi, b2, 127.0, 8388608.0,
                                 op0=AL.add, op1=AL.mult)
                ve.tensor_sub(b1, b1, b2)
                ve.tensor_scalar(b2, b1, _K2, _K1, op0=AL.mult, op1=AL.add)
                ve.tensor_mul(b2, b2, b1)
                ve.scalar_tensor_tensor(ot[:, :w], b2, _K0,
                                        bi.bitcast(f32),
                                        op0=AL.add, op1=AL.mult)
                for d0 in range(0, w, 1024):
                    dw = min(1024, w - d0)
                    dma_engines[qi % 2].dma_start(
                        out=out[g, :, i * SW + d0 : i * SW + d0 + dw],
                        in_=ot[:, d0 : d0 + dw],
                    )
                    qi += 1
    return nc


def _split3_bf16(x32):
    """Exact 3-term bf16 decomposition of fp32 (hi+mid+lo == x bit-exact)."""
    import ml_dtypes

    bf = ml_dtypes.bfloat16
    hi = x32.astype(bf)
    r1 = x32 - hi.astype(np.float32)
    mid = r1.astype(bf)
    r2 = r1 - mid.astype(np.float32)
    lo = r2.astype(bf)
    return np.stack([hi, mid, lo], axis=0)


def _pack_tau3(tau_flat):
    """[3, F] split -> [48, 2048]: row 3*s + t = term t of stripe s."""
    t3 = _split3_bf16(tau_flat)  # [3, F]
    return np.ascontiguousarray(
        t3.reshape(3, NST, CH).transpose(1, 0, 2).reshape(3 * NST, CH)
    )


def _stripe_mask():
    """[48, 16] bf16: mask[k, s] = 1.0 iff k // 3 == s."""
    import ml_dtypes

    m = np.zeros((3 * NST, NST), dtype=ml_dtypes.bfloat16)
    for s in range(NST):
        m[3 * s : 3 * (s + 1), s] = 1.0
    return np.ascontiguousarray(m)


_NC = None


def kernel(inputs, W, b, worker_num=WN, task_num=TN, edge_type=L, ability_num=A, **_kw):
    global _NC
    inputs = np.ascontiguousarray(np.asarray(inputs, dtype=np.float32))
    W = np.asarray(W, dtype=np.float32).reshape(A)
    b = np.asarray(b, dtype=np.float32).reshape(1)
    assert inputs.shape == (WN + TN, A)

    wf = inputs[:WN, :A]
    tau3 = _pack_tau3(inputs[WN:, :L].reshape(F))

    if _NC is None:
        _NC = build_nc()

    mask = _stripe_mask()
    in_maps = [
        {
            "wf": np.ascontiguousarray(wf[k * WPC : (k + 1) * WPC]),
            "tau3": tau3,
            "mask": mask,
            "W": W,
            "b": b,
        }
        for k in range(NCORES)
    ]
    res = run_bass_kernel_spmd(_NC, in_maps, core_ids=list(range(NCORES)))
    parts = [
        r["out"].astype(np.float32).reshape(WPC, TN, L) for r in res.results
    ]
    return np.concatenate(parts, axis=0)


# revision 71
# speedup vs baseline: 1.0092x; 1.0092x over previous
"""Trainium2 Bass kernel for nn_Decoder (worker/task label-probability decoder).

Math:
    worker_feature = inputs[:2048, :64]          # [Wn, A]
    tau            = inputs[2048:, :16]          # [T, L]
    p1 = sigmoid(worker_feature @ W + b)         # [Wn, 1]
    p2 = (1 - p1) / (L - 1)
    P[i, j, l] = p1[i]^tau[j,l] * p2[i]^(1 - tau[j,l])
               = exp(a[i] * tau[j,l] + c[i]),  a = ln p1 - ln p2, c = ln p2

Sharding: pure data parallel over the worker axis (dim 0), 256 workers per
core across 8 cores; tau/W/b replicated. No communication.

Per-core layout: workers on SBUF partitions (2 groups of 128), task axis
flattened on the free dimension. tau ships as an exact 3-term bf16 split,
striped [48 x 2048] so the load is one cheap DMA; the otherwise-idle tensor
engine replicates each stripe to all 128 partitions (ones.T @ tau3 sums the
three bf16 terms in fp32 -> exact tau in PSUM). The scalar engine computes
Exp(a*tau + c) with per-partition scale/bias, writing bf16 tiles (the 2e-2
rel-err budget dwarfs bf16's 2e-3 rounding), halving HBM write traffic.

The scalar engine is the roofline (~58 us of Exp incl. per-op overhead);
everything else is arranged to keep it saturated: the DVE drains each PSUM
piece into [128, 8192] f32 SBUF stages (so the 2-deep PSUM ring frees fast
and the PE never gates ACT) and ACT runs wide Exp ops out of SBUF; output
DMAs ride only the SP and Pool queues (a DMA costs its issuing engine
~0.39 ns per partition-byte); the Exp/Ln table is prefetched via a dummy
op during the input loads; dummy matmuls warm the PE p-state; and the
first/last stages are read in a copy-chasing ladder so pipeline fill and
drain stay short.
"""

import numpy as np

try:
    import concourse.bass as bass  # noqa: F401
except ImportError:  # fall back to the container's repo checkout
    import sys

    for _p in ("/root/.axon_site/_ro/trn_rl_repo", "/opt/trn_rl_repo"):
        if _p not in sys.path:
            sys.path.append(_p)

import concourse.bass as bass
import concourse.tile as tile
from concourse import library_config, mybir
from concourse.bass_utils import run_bass_kernel_spmd
from concourse.vector_clock import ScopedClock

WN = 2048  # workers total
TN = 2048  # tasks
L = 16  # edge types / labels
A = 64  # ability features
NCORES = 8
WPC = WN // NCORES  # workers per core (256)
G = WPC // 128  # partition groups per core (2)
F = TN * L  # flattened task axis (32768)
CH = 2048  # stripe length: PSUM ping-pong granularity + per-chunk output DMA
NST = F // CH  # tau stripes (16)
NWARM = 18  # dummy matmuls to warm the PE p-state

SW = 8192  # SBUF stage width: 4 PSUM pieces aggregated per stage
NSTG = F // SW  # stages (4)

_AF = mybir.ActivationFunctionType

# minimax-ish quadratic for 2^f on [-0.5, 0.5] (rel err ~2.4e-3)
_K0, _K1, _K2 = 1.0005544420658883, 0.7060853046731855, 0.23974148211482982
_LOG2E = 1.4426950408889634


class _TC(tile.TileContext):
    """TileContext legalized for a walrus that allows one sync-wait per inst.

    The walrus build in this container rejects any instruction carrying more
    than one sync-wait command. After Tile's normal scheduling + the exit
    drain/barrier, rewrite every multi-wait instruction into a chain of
    same-engine NOPs (one wait each) followed by the instruction with the
    final wait.
    """

    def _drain_and_barrier(self, tick_clock, wait_clock):
        super()._drain_and_barrier(tick_clock, wait_clock)
        self._split_multi_waits()

    def _fresh_nop(self, engine):
        inst = self.nc.engines[engine].nop(nofuse=True).ins
        self.nc.cur_bb.bb.instructions.remove(inst)
        return inst

    def _split_multi_waits(self):
        for fn in self.nc.m.functions:
            for bb in fn.blocks:
                snapshot = list(bb.instructions)
                if not any(
                    inst.sync_info and len(inst.sync_info.on_wait) > 1
                    for inst in snapshot
                ):
                    continue
                new = []
                for inst in snapshot:
                    si = inst.sync_info
                    if si is not None and si.on_wait and len(si.on_wait) > 1:
                        waits = list(si.on_wait)
                        si.on_wait = waits[-1:]
                        inst.sync_info = si
                        for wt in waits[:-1]:
                            nop = self._fresh_nop(inst.engine)
                            nop.sync_info = mybir.SyncInfo(on_wait=[wt], on_update=[])
                            new.append(nop)
                    new.append(inst)
                bb.instructions[:] = new


def build_nc():
    nc = bass.Bass("TRN2")
    wf = nc.dram_tensor("wf", [WPC, A], mybir.dt.float32, kind="ExternalInput")
    # tau3[3*s + t] = bf16 term t of tau stripe s (exact hi/mid/lo split).
    tau3_in = nc.dram_tensor(
        "tau3", [3 * NST, CH], mybir.dt.bfloat16, kind="ExternalInput"
    )
    # mask[k, s] = 1.0 iff row k belongs to stripe s. Used as matmul weights
    # via a stride-0 AP that broadcasts column s to all 128 weight columns,
    # so the contraction over all 48 rows extracts exactly stripe s's three
    # bf16 terms (their fp32 sum = tau, bit-exact).
    mask_in = nc.dram_tensor(
        "mask", [3 * NST, NST], mybir.dt.bfloat16, kind="ExternalInput"
    )
    w_in = nc.dram_tensor("W", [A], mybir.dt.float32, kind="ExternalInput")
    b_in = nc.dram_tensor("b", [1], mybir.dt.float32, kind="ExternalInput")
    out = nc.dram_tensor("out", [G, 128, F], mybir.dt.bfloat16, kind="ExternalOutput")

    f32 = mybir.dt.float32
    bf16 = mybir.dt.bfloat16

    with _TC(nc) as tc:
        with (
            tc.tile_pool(name="const", bufs=1) as const,
            tc.tile_pool(name="outs", bufs=3) as outs,
            tc.tile_pool(name="psum", bufs=2, space="PSUM") as psum,
        ):
            ones = const.tile([3, 128], bf16)
            nc.vector.memset(ones, 1.0)

            # Prefetch the Exp/Ln activation table while inputs load: the
            # first table load costs 1.28 us and must not sit on the
            # prologue's critical path.
            scr_act = const.tile([3, 128], bf16)
            nc.scalar.activation(scr_act, ones, _AF.Exp)

            # Warm the PE p-state: dummy matmuls keep the tensor engine
            # continuously busy from t~0 so the real chunk-0 matmuls run at
            # full clock instead of the 0.65 GHz cold state.
            for wi in range(NWARM):
                wpt = psum.tile([128, CH], f32, tag="pt", name=f"warm{wi}")
                nc.tensor.matmul(
                    wpt[:, 0:128], ones, ones, start=True, stop=True
                )

            # ---- input loads: wf/W/b on SP, tau3 on Pool (one 4KB/partition
            # DMA); the ACT ring never carries a DMA.
            wf_sb = const.tile([128, G, A], f32)
            nc.sync.dma_start(
                out=wf_sb, in_=wf[:].rearrange("(g p) a -> p g a", p=128)
            )
            w_ap = w_in[:]
            w_sb = const.tile([128, A], f32)
            nc.sync.dma_start(
                out=w_sb,
                in_=bass.AP(tensor=w_ap.tensor, offset=w_ap.offset, ap=[[0, 128], [1, A]]),
            )
            b_ap = b_in[:]
            b_sb = const.tile([128, 1], f32)
            nc.sync.dma_start(
                out=b_sb,
                in_=bass.AP(tensor=b_ap.tensor, offset=b_ap.offset, ap=[[0, 128], [1, 1]]),
            )
            # tau loaded in column pieces so stripe 0's early matmuls (which
            # only read the first columns) can start ~2.3us in; the mask is a
            # single tiny load.
            tau_sb = const.tile([3 * NST, CH], bf16)
            mask_sb = const.tile([3 * NST, NST], bf16)
            nc.gpsimd.dma_start(out=tau_sb[:, :512], in_=tau3_in[:, :512])
            nc.gpsimd.dma_start(out=mask_sb, in_=mask_in[:])
            nc.gpsimd.dma_start(out=tau_sb[:, 512:1024], in_=tau3_in[:, 512:1024])
            nc.gpsimd.dma_start(out=tau_sb[:, 1024:], in_=tau3_in[:, 1024:])

            def stripe_w(s):
                col = mask_sb[:, s : s + 1]
                return bass.AP(
                    tensor=col.tensor, offset=col.offset, ap=[[col.ap[0][0], 3 * NST], [0, 128]]
                )

            # ---- per-worker scalars (on GPSIMD, so the DVE is free to run
            # the PSUM->SBUF copy pipeline from t~3us):
            #   x' = wf.W + b;  a = ln p1 - ln p2 = x' + ln15  (exact identity)
            #   c  = ln p2 = -x' - ln15 - ln(1 + e^{-x'})
            LN15 = float(np.log(np.float32(L - 1)))
            x = const.tile([128, G], f32)
            for g in range(G):
                prod = const.tile([128, A], f32, tag=f"prod{g}")
                nc.vector.tensor_mul(prod, wf_sb[:, g, :], w_sb)
                nc.vector.reduce_sum(x[:, g : g + 1], prod, axis=mybir.AxisListType.X)

            pb = const.tile([128, 1], f32)  # b + ln15
            nc.vector.tensor_scalar_add(pb, b_sb, LN15)
            bneg = const.tile([128, 1], f32)
            nc.vector.tensor_scalar_mul(bneg, b_sb, -1.0)
            a_sb = const.tile([128, G], f32)
            nc.gpsimd.tensor_scalar_add(a_sb, x, pb[:, 0:1])
            # e = exp(-(x + b));  u = ln(1 + e);  c = -(u + x + b + ln15)
            e = const.tile([128, G], f32)
            nc.scalar.activation(e, x, _AF.Exp, bias=bneg[:, 0:1], scale=-1.0)
            u = const.tile([128, G], f32)
            nc.scalar.activation(u, e, _AF.Ln, bias=1.0, scale=1.0)
            s1 = const.tile([128, G], f32)
            nc.gpsimd.tensor_add(s1, u, x)
            lp2 = const.tile([128, G], f32)  # = c
            nc.gpsimd.tensor_scalar(
                lp2,
                s1,
                scalar1=pb[:, 0:1],
                scalar2=-1.0,
                op0=mybir.AluOpType.add,
                op1=mybir.AluOpType.mult,
            )
            # log2-scaled scale/bias for the DVE exp2 chunk
            ap_sb = const.tile([128, G], f32)
            nc.gpsimd.tensor_scalar_mul(ap_sb, a_sb, _LOG2E)
            cp_sb = const.tile([128, G], f32)
            nc.gpsimd.tensor_scalar_mul(cp_sb, lp2, _LOG2E)
            # DVE exp2 scratch
            xb1 = const.tile([128, CH], f32)
            xb2 = const.tile([128, CH], f32)
            xbi = const.tile([128, CH], mybir.dt.int32)
            gb1 = const.tile([128, 1024], f32)
            gb2 = const.tile([128, 1024], f32)
            gbi = const.tile([128, 1024], mybir.dt.int32)
            gb3 = const.tile([128, 1024], f32)

            # ---- main loop ----
            # PE broadcasts tau stripe s into a PSUM piece; the otherwise-idle
            # DVE drains each piece to a [128, 8192] f32 SBUF stage (freeing
            # the 2-deep PSUM ring early, so the PE never gates the scalar
            # engine); ACT then runs big 8192-wide Exp ops out of SBUF (fewer
            # per-instruction overheads) and the bf16 result streams to HBM
            # in 2048-chunks alternating between the SP and Pool queues.
            dma_engines = [nc.sync, nc.gpsimd]
            qi = 0

            # Work queue: stripe 0 split into 3 column-pieces (matching the
            # tau load ladder) so the copy pipeline starts with the first
            # 512 columns; each piece gets its own PSUM tile so a copy only
            # waits for the matmuls it actually covers.
            pieces = [(0, 0, 512), (0, 512, 512), (0, 1024, 1024)] + [
                (s, 0, CH) for s in range(1, NST)
            ]

            def stage_plan(i):
                # Stage 0: group 0 chases the copy pipeline piece-by-piece
                # (ACT's 2 passes/element outrun the copy's ~1.04
                # ns/element); group 1 follows in two 4096 reads. Last
                # stage: the final ACT is small so the drain tail is short.
                if i == 0:
                    g0 = [(0, 512), (512, 512), (1024, 1024), (2048, 2048),
                          (4096, 2048), (6144, 2048)]
                    return [(g, off, w) for off, w in g0 for g in range(G)]
                if i == 1:
                    # DVE hasn't caught up yet when ACT reaches stage 1: let
                    # the first reader chase per-piece, second reads whole.
                    return [(0, 0, CH), (0, CH, CH), (0, 2 * CH, 2 * CH)] + [(1, 1024, SW - 1024)]
                if i == 2:
                    # g1's first 1024 run on the DVE (exp2 bit-trick)
                    return [(0, 0, SW), (1, 1024, SW - 1024)]
                if i == NSTG - 1:
                    # g1's first 2048 run on the DVE (exp2 bit-trick), not ACT
                    g1 = [(1536, 2048), (3584, 2048),
                          (5632, 2048), (7680, 512)]
                    return [(0, 0, SW)] + [(1, off, w) for off, w in g1]
                return [(g, 0, SW) for g in range(G)]

            stgs, otss = {}, {}
            stg = None
            cum = 0
            for p, (s, c0, w) in enumerate(pieces):
                pt = psum.tile([128, CH], f32, tag="pt", name=f"pt{p}")
                for n in range(0, w, 512):
                    nc.tensor.matmul(
                        pt[:, n : n + 512],
                        stripe_w(s),
                        tau_sb[:, c0 + n : c0 + n + 512],
                        start=True,
                        stop=True,
                    )
                i, off = divmod(cum, SW)
                if off == 0:
                    stg = outs.tile([128, SW], f32, tag="stg", name=f"stg{i}", bufs=3)
                    stgs[i] = stg
                nc.vector.tensor_scalar_mul(
                    stg[:, off : off + w], pt[:, :w], 1.0
                )
                cum += w
                if cum % SW != 0:
                    continue
                ots = [
                    outs.tile([128, SW], bf16, tag=f"ot{g}", name=f"ot{g}_s{i}", bufs=2)
                    for g in range(G)
                ]
                otss[i] = ots
                if i == 2:
                    # GPSIMD exp2 chunk for stage1-g1[0:1024]: the 6 plain
                    # f32 ops encode on Pool; the int32-converting op and the
                    # bitcast-reading stt run on DVE (emitted post-loop).
                    AL = mybir.AluOpType
                    KM = float(1.5 * 2**23)
                    gp = nc.gpsimd
                    st1 = stgs[1]
                    gp.tensor_scalar(gb1, st1[:, :1024], ap_sb[:, 1:2],
                                     cp_sb[:, 1:2], op0=AL.mult, op1=AL.add)
                    gp.tensor_scalar(gb2, gb1, KM, None, op0=AL.add)
                    gp.tensor_scalar(gb2, gb2, -KM, None, op0=AL.add)
                    gp.tensor_sub(gb1, gb1, gb2)
                    gp.tensor_scalar(gb3, gb1, _K2, _K1, op0=AL.mult, op1=AL.add)
                    gp.tensor_mul(gb3, gb3, gb1)
                for g, off, w in stage_plan(i):
                    nc.scalar.activation(
                        ots[g][:, off : off + w],
                        stg[:, off : off + w],
                        _AF.Exp,
                        bias=lp2[:, g : g + 1],
                        scale=a_sb[:, g : g + 1],
                    )
                    for d0 in range(off, off + w, CH):
                        dw = min(CH, off + w - d0)
                        dma_engines[qi % 2].dma_start(
                            out=out[g, :, i * SW + d0 : i * SW + d0 + dw],
                            in_=ots[g][:, d0 : d0 + dw],
                        )
                        qi += 1

            # DVE exp2 bit-trick chunks + the two DVE-only hops of the
            # GPSIMD chunk (f32->int32 convert and bitcast-reading stt).
            AL = mybir.AluOpType
            KM = float(1.5 * 2**23)
            ve = nc.vector
            ve.tensor_scalar(gbi, gb2, 127.0, 8388608.0,
                             op0=AL.add, op1=AL.mult)  # GP chunk exp bits
            # wait: gb2 holds n only until GP's q overwrites it; the Tile
            # dependency on gb2's WAR serializes GP's q behind this op.
            for ci, (i, g, w) in enumerate([(2, 1, 1024)]):
                stg, ot = stgs[i], otss[i][g]
                b1, b2, bi = xb1[:, :w], xb2[:, :w], xbi[:, :w]
                ve.tensor_scalar(b1, stg[:, :w], ap_sb[:, g : g + 1],
                                 cp_sb[:, g : g + 1], op0=AL.mult, op1=AL.add)
                ve.tensor_scalar(b2, b1, KM, None, op0=AL.add)
                ve.tensor_scalar(b2, b2, -KM, None, op0=AL.add)
                ve.tensor_scalar(bi, b2, 127.0, 8388608.0,
                                 op0=AL.add, op1=AL.mult)
                ve.tensor_sub(b1, b1, b2)
                ve.tensor_scalar(b2, b1, _K2, _K1, op0=AL.mult, op1=AL.add)
                ve.tensor_mul(b2, b2, b1)
                ve.scalar_tensor_tensor(ot[:, :w], b2, _K0,
                                        bi.bitcast(f32),
                                        op0=AL.add, op1=AL.mult)
                for d0 in range(0, w, 1024):
                    dma_engines[qi % 2].dma_start(
                        out=out[g, :, i * SW + d0 : i * SW + d0 + 1024],
                        in_=ot[:, d0 : d0 + 1024],
                    )
                    qi += 1
            # GP chunk finish: (m + k0) * 2^n -> ot (stage1 g1 [0:1024])
            ve.scalar_tensor_tensor(otss[1][1][:, :1024], gb3, _K0,
                                    gbi.bitcast(f32), op0=AL.add, op1=AL.mult)
            nc.gpsimd.dma_start(
                out=out[1, :, SW : SW + 1024], in_=otss[1][1][:, :1024]
            )
            # DVE chunk1: stage3 g1 [0:1536]
            for ci, (i, g, w) in enumerate([(3, 1, 1536)]):
                stg, ot = stgs[i], otss[i][g]
                b1, b2, bi = xb1[:, :w], xb2[:, :w], xbi[:, :w]
                ve.tensor_scalar(b1, stg[:, :w], ap_sb[:, g : g + 1],
                                 cp_sb[:, g : g + 1], op0=AL.mult, op1=AL.add)
                ve.tensor_scalar(b2, b1, KM, None, op0=AL.add)
                ve.tensor_scalar(b2, b2, -KM, None, op0=AL.add)
                ve.tensor_scalar(bi, b2, 127.0, 8388608.0,
                                 op0=AL.add, op1=AL.mult)
                ve.tensor_sub(b1, b1, b2)
                ve.tensor_scalar(b2, b1, _K2, _K1, op0=AL.mult, op1=AL.add)
                ve.tensor_mul(b2, b2, b1)
                ve.scalar_tensor_tensor(ot[:, :w], b2, _K0,
                                        bi.bitcast(f32),
                                        op0=AL.add, op1=AL.mult)
                for d0 in range(0, w, 1024):
                    dw = min(1024, w - d0)
                    dma_engines[qi % 2].dma_start(
                        out=out[g, :, i * SW + d0 : i * SW + d0 + dw],
                        in_=ot[:, d0 : d0 + dw],
                    )
                    qi += 1
    return nc


def _split3_bf16(x32):
    """Exact 3-term bf16 decomposition of fp32 (hi+mid+lo == x bit-exact)."""
    import ml_dtypes

    bf = ml_dtypes.bfloat16
    hi = x32.astype(bf)
    r1 = x32 - hi.astype(np.float32)
    mid = r1.astype(bf)
    r2 = r1 - mid.astype(np.float32)
    lo = r2.astype(bf)
    return np.stack([hi, mid, lo], axis=0)


def _pack_tau3(tau_flat):
    """[3, F] split -> [48, 2048]: row 3*s + t = term t of stripe s."""
    t3 = _split3_bf16(tau_flat)  # [3, F]
    return np.ascontiguousarray(
        t3.reshape(3, NST, CH).transpose(1, 0, 2).reshape(3 * NST, CH)
    )


def _stripe_mask():
    """[48, 16] bf16: mask[k, s] = 1.0 iff k // 3 == s."""
    import ml_dtypes

    m = np.zeros((3 * NST, NST), dtype=ml_dtypes.bfloat16)
    for s in range(NST):
        m[3 * s : 3 * (s + 1), s] = 1.0
    return np.ascontiguousarray(m)


_NC = None


def kernel(inputs, W, b, worker_num=WN, task_num=TN, edge_type=L, ability_num=A, **_kw):
    global _NC
    inputs = np.ascontiguousarray(np.asarray(inputs, dtype=np.float32))
    W = np.asarray(W, dtype=np.float32).reshape(A)
    b = np.asarray(b, dtype=np.float32).reshape(1)
    assert inputs.shape == (WN + TN, A)

    wf = inputs[:WN, :A]
    tau3 = _pack_tau3(inputs[WN:, :L].reshape(F))

    if _NC is None:
        _NC = build_nc()

    mask = _stripe_mask()
    in_maps = [
        {
            "wf": np.ascontiguousarray(wf[k * WPC : (k + 1) * WPC]),
            "tau3": tau3,
            "mask": mask,
            "W": W,
            "b": b,
        }
        for k in range(NCORES)
    ]
    res = run_bass_kernel_spmd(_NC, in_maps, core_ids=list(range(NCORES)))
    parts = [
        r["out"].astype(np.float32).reshape(WPC, TN, L) for r in res.results
    ]
    return np.concatenate(parts, axis=0)
